# revision 1
# baseline (speedup 1.0000x reference)
"""GQA kernel for Trainium2, sharded over 8 NeuronCores.

Sharding: tensor-parallel over heads. Core g owns Q heads 4g..4g+3 and KV
group g (GQA rep=4, so all 4 local heads share one K/V). The reference's
final projection contracts over the *sequence* axis (faithful swapaxes
quirk), so output rows partition cleanly by head: core g produces rows
g*256..(g+1)*256 of the [2, 2048, 2048] output. No collectives.

Device dataflow per core, per batch b:
  XT = X[b].T (host-prepped, bf16)    [hidden, seq]
  QT = wq.T @ XT   (PE, psum accum over hidden chunks)   [256, 2048]
  KVT = wkv.T @ XT                                        [128, 2048]
  V  = transpose(KVT[64:128]) via PE identity-matmul, plus ones column
  per head h: scoresT[k,q] = KT.T @ QT_h  (K=64 contraction)
              probsT = exp(scoresT)  (ACT, scale folded into wq on host)
              avT[65, q] = V'.T @ probsT  (accum over k chunks; row 64 =
                                           softmax denominator)
              attn[q, d] = transpose(avT) * (1/denominator)  per 128-q block
  out rows = attn_nat.T @ wo  (contract over q/seq), + b_o
"""
import numpy as np
import ml_dtypes

import concourse.bass as bass
import concourse.bacc as bacc
import concourse.mybir as mybir
import concourse.tile as tile
from concourse import bass_utils
from concourse.masks import make_identity

BF16 = mybir.dt.bfloat16
F32 = mybir.dt.float32
NP_BF16 = ml_dtypes.bfloat16

B, S, HID = 2, 2048, 2048
NCORES = 8
HEADS_PER_CORE = 4   # of 32
D = 64               # head dim
QF = HEADS_PER_CORE * D   # 256 q-features per core
P = 128
HC = HID // P        # 16 hidden chunks
SC = S // P          # 16 seq chunks

_CACHE = {}


def _build():
    nc = bacc.Bacc("TRN2", target_bir_lowering=False, debug=False,
                   num_devices=NCORES)
    # ---- DRAM I/O ----
    xt_d = nc.dram_tensor("xt", [B, HID, S], BF16, kind="ExternalInput").ap()
    wq_d = nc.dram_tensor("wq", [HID, QF], BF16, kind="ExternalInput").ap()
    bq_d = nc.dram_tensor("bq", [2, P], F32, kind="ExternalInput").ap()
    wkv_d = nc.dram_tensor("wkv", [HID, P], BF16, kind="ExternalInput").ap()
    bkv_d = nc.dram_tensor("bkv", [P, 1], F32, kind="ExternalInput").ap()
    wo_d = nc.dram_tensor("wo", [HID, HID], BF16, kind="ExternalInput").ap()
    bo_d = nc.dram_tensor("bo", [P, HID], F32, kind="ExternalInput").ap()
    out_d = nc.dram_tensor("out", [B, QF, HID], F32, kind="ExternalOutput").ap()

    with tile.TileContext(nc) as tc:
        with (
            tc.tile_pool(name="consts", bufs=1) as consts,
            tc.tile_pool(name="xt", bufs=1) as xt_pool,
            tc.tile_pool(name="qt", bufs=2) as qt_pool,
            tc.tile_pool(name="kvt", bufs=1) as kvt_pool,
            tc.tile_pool(name="vp", bufs=2) as vp_pool,
            tc.tile_pool(name="pt", bufs=2) as pt_pool,
            tc.tile_pool(name="attnT", bufs=1) as attnT_pool,
            tc.tile_pool(name="attn", bufs=2) as attn_pool,
            tc.tile_pool(name="wos", bufs=1) as wos_pool,
            tc.tile_pool(name="outp", bufs=2) as out_pool,
            tc.tile_pool(name="rcp", bufs=4) as rcp_pool,
            tc.tile_pool(name="psum", bufs=1, space="PSUM") as psum,
        ):
            # ---- constants ----
            wq_sb = consts.tile([P, HC, QF], BF16)
            nc.sync.dma_start(wq_sb, wq_d.rearrange("(hc p) q -> p hc q", p=P))
            wkv_sb = consts.tile([P, HC, P], BF16)
            nc.sync.dma_start(wkv_sb, wkv_d.rearrange("(hc p) q -> p hc q", p=P))
            bq_sb = consts.tile([P, 2], F32)
            nc.sync.dma_start(bq_sb, bq_d.rearrange("c p -> p c"))
            bkv_sb = consts.tile([P, 1], F32)
            nc.sync.dma_start(bkv_sb, bkv_d)
            bo_sb = consts.tile([P, HID], F32)
            nc.sync.dma_start(bo_sb, bo_d)
            ident = consts.tile([P, P], BF16)
            make_identity(nc, ident)
            # shifted identity block at partitions 64-127, cols 0-63 (so the
            # V-transpose operands share base_partition 64)
            ident2 = consts.tile([P, P], BF16)
            nc.sync.dma_start(ident2[64:128, 0:64], ident[0:64, 0:64])

            attn_nat = {}   # per-batch normalized attention, [q, (h d)] bf16

            for b in range(B):
                # ---- load XT[b] ----
                xt_sb = xt_pool.tile([P, HC, S], BF16, tag="xt")
                for hc in range(HC):
                    nc.sync.dma_start(xt_sb[:, hc, :],
                                      xt_d[b, hc * P:(hc + 1) * P, :])

                # ---- QT projection: [256, 2048] ----
                qt_sb = qt_pool.tile([P, 2, S], BF16, tag="qt")
                for qc in range(2):
                    for sh in range(2):
                        ps = psum.tile([P, 1024], F32, tag="sc", bufs=2)
                        for j in range(2):
                            for hc in range(HC):
                                nc.tensor.matmul(
                                    ps[:, j * 512:(j + 1) * 512],
                                    lhsT=wq_sb[:, hc, qc * P:(qc + 1) * P],
                                    rhs=xt_sb[:, hc, sh * 1024 + j * 512:
                                              sh * 1024 + (j + 1) * 512],
                                    start=(hc == 0), stop=(hc == HC - 1))
                        nc.vector.tensor_tensor(
                            out=qt_sb[:, qc, sh * 1024:(sh + 1) * 1024],
                            in0=ps, in1=bq_sb[:, qc:qc + 1].to_broadcast((P, 1024)),
                            op=mybir.AluOpType.add)

                # ---- KVT projection: [128, 2048] (K rows 0-63, V rows 64-127)
                kvt_sb = kvt_pool.tile([P, S], BF16, tag="kvt")
                for sh in range(2):
                    ps = psum.tile([P, 1024], F32, tag="sc", bufs=2)
                    for j in range(2):
                        for hc in range(HC):
                            nc.tensor.matmul(
                                ps[:, j * 512:(j + 1) * 512],
                                lhsT=wkv_sb[:, hc, :],
                                rhs=xt_sb[:, hc, sh * 1024 + j * 512:
                                          sh * 1024 + (j + 1) * 512],
                                start=(hc == 0), stop=(hc == HC - 1))
                    nc.vector.tensor_tensor(
                        out=kvt_sb[:, sh * 1024:(sh + 1) * 1024],
                        in0=ps, in1=bkv_sb[:, 0:1].to_broadcast((P, 1024)),
                        op=mybir.AluOpType.add)

                # replicate KT into partitions 64-127 so odd heads' scores
                # matmuls have matching operand base partitions
                kt2_sb = kvt_pool.tile([P, S], BF16, tag="kt2")
                nc.sync.dma_start(kt2_sb[64:128, :], kvt_sb[0:64, :])

                # ---- V natural + ones column: [k, 65] per k-chunk ----
                vp_sb = vp_pool.tile([P, SC, 65], BF16, tag="vp")
                nc.vector.memset(vp_sb[:, :, 64], 1.0)
                for kc in range(SC):
                    tr = psum.tile([P, 64], BF16, tag="tr", bufs=2)
                    nc.tensor.transpose(
                        tr, kvt_sb[64:128, kc * P:(kc + 1) * P],
                        ident2[64:128, 0:64])
                    nc.vector.tensor_copy(out=vp_sb[:, kc, 0:64], in_=tr)

                # ---- attention per local head ----
                attn_sb = attn_pool.tile([P, SC, QF], BF16, tag="attn")
                attn_nat[b] = attn_sb
                for h in range(4):
                    pbase = (h % 2) * 64
                    qt_h = qt_sb[pbase:pbase + 64, h // 2, :]   # [64, 2048]
                    kt_h = (kvt_sb if h % 2 == 0 else kt2_sb)[pbase:pbase + 64, :]
                    attnT_sb = attnT_pool.tile([65, S], BF16, tag="attnT")
                    for qtp in range(2):
                        av = psum.tile([P, 1024], F32, tag="av", bufs=1)
                        for kc in range(SC):
                            sc_ps = psum.tile([P, 1024], F32, tag="sc", bufs=2)
                            for j in range(2):
                                nc.tensor.matmul(
                                    sc_ps[:, j * 512:(j + 1) * 512],
                                    lhsT=kt_h[:, kc * P:(kc + 1) * P],
                                    rhs=qt_h[:, qtp * 1024 + j * 512:
                                             qtp * 1024 + (j + 1) * 512],
                                    start=True, stop=True)
                            pt = pt_pool.tile([P, 1024], BF16, tag="pt")
                            nc.scalar.activation(
                                pt, sc_ps, mybir.ActivationFunctionType.Exp)
                            for j in range(2):
                                nc.tensor.matmul(
                                    av[0:65, j * 512:(j + 1) * 512],
                                    lhsT=vp_sb[:, kc, :],
                                    rhs=pt[:, j * 512:(j + 1) * 512],
                                    start=(kc == 0), stop=(kc == SC - 1),
                                    skip_group_check=True)
                        nc.vector.tensor_copy(
                            out=attnT_sb[:, qtp * 1024:(qtp + 1) * 1024],
                            in_=av[0:65, :])
                    # transpose + normalize into attn_nat[:, :, h*64:(h+1)*64]
                    for tb in range(SC):
                        tr2 = psum.tile([P, 65], BF16, tag="tr", bufs=2)
                        nc.tensor.transpose(
                            tr2, attnT_sb[:, tb * P:(tb + 1) * P],
                            ident[0:65, 0:65])
                        rcp = rcp_pool.tile([P, 1], F32, tag="rcp")
                        nc.vector.reciprocal(rcp, tr2[:, 64:65])
                        nc.vector.tensor_tensor(
                            out=attn_sb[:, tb, h * D:(h + 1) * D],
                            in0=tr2[:, 0:64],
                            in1=rcp.to_broadcast((P, 64)),
                            op=mybir.AluOpType.mult)

            # ---- final projection: out[b, r, :] = attn_nat.T @ wo + bo ----
            for jh in range(2):
                wo_sl = wos_pool.tile([P, SC, 1024], BF16, tag="wo")
                for sq in range(SC):
                    nc.sync.dma_start(
                        wo_sl[:, sq, :],
                        wo_d[sq * P:(sq + 1) * P, jh * 1024:(jh + 1) * 1024])
                for b in range(B):
                    for rc in range(2):
                        ps = psum.tile([P, 1024], F32, tag="sc", bufs=2)
                        for j in range(2):
                            for sq in range(SC):
                                nc.tensor.matmul(
                                    ps[:, j * 512:(j + 1) * 512],
                                    lhsT=attn_nat[b][:, sq, rc * P:(rc + 1) * P],
                                    rhs=wo_sl[:, sq, j * 512:(j + 1) * 512],
                                    start=(sq == 0), stop=(sq == SC - 1))
                        out_sb = out_pool.tile([P, 1024], F32, tag="out")
                        nc.vector.tensor_tensor(
                            out=out_sb, in0=ps,
                            in1=bo_sb[:, jh * 1024:(jh + 1) * 1024],
                            op=mybir.AluOpType.add)
                        nc.sync.dma_start(
                            out_d[b, rc * P:(rc + 1) * P,
                                  jh * 1024:(jh + 1) * 1024],
                            out_sb)

    nc.compile()
    return nc


def _get_nc():
    if "nc" not in _CACHE:
        _CACHE["nc"] = _build()
    return _CACHE["nc"]


def _prep_inputs(hidden_state, w_q, b_q, w_k, b_k, w_v, b_v, w_o, b_o):
    """Host-side sharding/layout prep. Only layout/dtype transforms."""
    xt = np.ascontiguousarray(hidden_state.transpose(0, 2, 1)).astype(NP_BF16)
    wo = np.ascontiguousarray(w_o).astype(NP_BF16)
    bo = np.broadcast_to(b_o.astype(np.float32), (P, HID)).copy()
    in_maps = []
    for g in range(NCORES):
        wq_g = np.ascontiguousarray(
            w_q[:, g * QF:(g + 1) * QF] * 0.125).astype(NP_BF16)
        bq_g = np.ascontiguousarray(
            (b_q[g * QF:(g + 1) * QF] * 0.125).reshape(2, P)).astype(np.float32)
        wkv_g = np.ascontiguousarray(np.concatenate(
            [w_k[:, g * D:(g + 1) * D], w_v[:, g * D:(g + 1) * D]],
            axis=1)).astype(NP_BF16)
        bkv_g = np.ascontiguousarray(np.concatenate(
            [b_k[g * D:(g + 1) * D], b_v[g * D:(g + 1) * D]])
            .reshape(P, 1)).astype(np.float32)
        in_maps.append({
            "xt": xt, "wq": wq_g, "bq": bq_g, "wkv": wkv_g, "bkv": bkv_g,
            "wo": wo, "bo": bo,
        })
    return in_maps


def kernel(hidden_state, w_q, b_q, w_k, b_k, w_v, b_v, w_o, b_o,
           _trace=False):
    hidden_state = np.asarray(hidden_state, np.float32)
    args = [np.asarray(a, np.float32) for a in
            (w_q, b_q, w_k, b_k, w_v, b_v, w_o, b_o)]
    nc = _get_nc()
    in_maps = _prep_inputs(hidden_state, *args)
    res = bass_utils.run_bass_kernel_spmd(
        nc, in_maps, core_ids=list(range(NCORES)), trace=_trace)
    out = np.concatenate([res.results[g]["out"] for g in range(NCORES)],
                         axis=1).astype(np.float32)
    if _trace:
        _CACHE["last_results"] = res
    return out

